# revision 30
# baseline (speedup 1.0000x reference)
"""HRNN (L=3, H=512) Trainium2 Bass kernel.

Strategy: data-parallel over batch (32 -> 4 rows/core on 8 cores).
Per core everything lives in a permuted H-major ("T128") SBUF layout:
slot (p=32q+i, group t) holds gate/hidden index 32*(4t+q)+i, batch on a
4-wide free dim.  Gate matmuls run batch-major (stationary = activation
k-tiles [128,4], moving = resident bf16 weights, PSUM B-major) and are
rotated into T128 by DVE 32x32 stream-transposes; weight rows/cols are
pre-permuted on the host so no other shuffles are needed.  x @ Wih0^T for
all timesteps is computed on-device in a prologue (weights-stationary, so
it lands H-major directly) and kept in SBUF as bf16.

v2: gate PSUM split into rz/n halves (ping-pong with DVE transposes),
elementwise chains spread across DVE/Pool/ACT, mixing coefficients
computed on [1,BL] tiles then broadcast once via PE, HT2/HB2 alias the
previous step's CT2/CB2, MLP matmuls interleave under GRU math.
"""

import os
import sys
from contextlib import ExitStack

import numpy as np

sys.path.insert(0, "/opt/trn_rl_repo")

import ml_dtypes  # noqa: E402

BF16 = ml_dtypes.bfloat16

L, H, B, T, D, NCORES = 3, 512, 32, 512, 768, 8
BL = B // NCORES          # 4 batch rows per core
G = 3 * H                 # 1536 gates
NT = G // 128             # 12 T128 groups
NU = H // 128             # 4 h k-tiles


def _gate_perm():
    # perm[t*128 + p] = gate index at T128 slot (p, t)
    p = np.arange(128)
    q, i = p // 32, p % 32
    out = np.zeros(G, np.int64)
    for t in range(NT):
        out[t * 128: (t + 1) * 128] = 32 * (4 * t + q) + i
    return out


def _h_perm():
    p = np.arange(128)
    q, i = p // 32, p % 32
    out = np.zeros(H, np.int64)
    for u in range(NU):
        out[u * 128: (u + 1) * 128] = 32 * (4 * u + q) + i
    return out


GPERM = _gate_perm()
HPERM = _h_perm()


def _mov(WT_perm, nk, n):
    # (nk*128, n) row-permuted W^T -> (128, nk, n) moving-weight layout
    return np.ascontiguousarray(
        WT_perm.reshape(nk, 128, n).transpose(1, 0, 2)).astype(BF16)


def _t128_vec(v):
    # (G,) or (H,) vector -> (128, ngroups) T128 grid
    perm = GPERM if v.shape[0] == G else HPERM
    n = v.shape[0] // 128
    return np.ascontiguousarray(
        v[perm].reshape(n, 128).T).astype(np.float32)


def prep_host(inp):
    """Build the per-core / shared device tensors from the raw inputs."""
    f32 = np.float32
    W = {}
    # Phase A stationary: Wih0 cols gate-permuted, lhsT layout
    WihA = inp["Wih0"][GPERM].T.astype(f32)            # (768, 1536)
    W["wihA"] = np.ascontiguousarray(
        WihA.reshape(6, 128, NT, 128).transpose(1, 0, 2, 3)).astype(BF16)
    # GRU0 moving: Whh0^T rows h-permuted
    W["w0"] = _mov(inp["Whh0"].T[HPERM], NU, G)
    # GRU1/2 moving: [Wih_l^T; Whh_l^T] rows h-permuted per half
    for l in (1, 2):
        cat = np.concatenate([inp["Wih"][l - 1].T[HPERM],
                              inp["Whh"][l - 1].T[HPERM]], axis=0)
        W[f"w{l}cat"] = _mov(cat, 2 * NU, G)
    # MLP moving weights (batch-major) + W3 stationary
    for l in (0, 1):
        W[f"mw1_{l}"] = _mov(inp["mW1"][l].T[HPERM], NU, H)
        W[f"mw2_{l}"] = _mov(inp["mW2"][l].T[HPERM], NU, H)
        w3 = inp["mW3"][l][0][HPERM].astype(f32)       # (512,)
        W[f"mw3_{l}"] = np.ascontiguousarray(
            w3.reshape(NU, 128).T[:, :, None]).astype(BF16)  # (128,4,1)
    # biases
    bA = inp["bih0"].astype(f32).copy()
    bA[:1024] += inp["bhh0"][:1024].astype(f32)
    W["biasA"] = _t128_vec(bA)                          # (128, 12)
    W["bhhn0"] = _t128_vec(inp["bhh0"][1024:].astype(f32))   # (128, 4)
    for l in (1, 2):
        brz = (inp["bih"][l - 1][:1024] + inp["bhh"][l - 1][:1024]).astype(f32)
        W[f"brz{l}"] = _t128_vec(np.concatenate([brz, np.zeros(512, f32)]))[:, :8]
        W[f"bihn{l}"] = _t128_vec(inp["bih"][l - 1][1024:].astype(f32))
        W[f"bhhn{l}"] = _t128_vec(inp["bhh"][l - 1][1024:].astype(f32))
    for l in (0, 1):
        W[f"mb1_{l}"] = _t128_vec(inp["mb1"][l].astype(f32))  # (128,4)
        W[f"mb2_{l}"] = _t128_vec(inp["mb2"][l].astype(f32))
        W[f"mb3_{l}"] = np.float32(inp["mb3"][l][0])
    return W


def prep_x(x_core):
    # (BL, T, D) -> (128, 6, T*BL) bf16, free order (step, batch)
    xt = x_core.transpose(2, 1, 0).reshape(D, T * BL)
    return np.ascontiguousarray(
        xt.reshape(6, 128, T * BL).transpose(1, 0, 2)).astype(BF16)


# --------------------------------------------------------------------------
# numpy emulator of the device program (layout validation)
# --------------------------------------------------------------------------

def emulate_core(W, x_core, t_steps=T):
    f32 = np.float32

    def mm_bmajor(stat_list, mov):
        # stat_list: list of (128, BL) k-tiles (bf16); mov: (128, nk, N)
        acc = np.zeros((BL, mov.shape[2]), f32)
        for u, s in enumerate(stat_list):
            acc += s.astype(f32).T @ mov[:, u, :].astype(f32)
        return acc

    def t128(bmaj):
        # (BL, N) plain gate order -> (128, N//128, 4) compact T128
        N = bmaj.shape[1]
        return np.ascontiguousarray(
            bmaj.reshape(BL, N // 128, 4, 32).transpose(2, 3, 1, 0)
            .reshape(128, N // 128, BL)).astype(f32)

    def stat_tiles(ht_b):  # (128, nu, BL) bf16 -> list of (128, BL)
        return [ht_b[:, u, :] for u in range(ht_b.shape[1])]

    xT = prep_x(x_core)  # (128, 6, T*BL) bf16
    # Phase A: gi0 (128, T, 12, 4) bf16
    gi0 = np.zeros((128, t_steps, NT, BL), BF16)
    for t in range(NT):
        acc = np.zeros((128, t_steps * BL), f32)
        for kt in range(6):
            acc += W["wihA"][:, kt, t, :].astype(f32).T @ \
                xT[:, kt, :t_steps * BL].astype(f32)
        acc += W["biasA"][:, t][:, None]
        gi0[:, :, t, :] = acc.reshape(128, t_steps, BL).astype(BF16)

    HT = [np.zeros((128, NU, BL), f32) for _ in range(L)]
    HB = [h.astype(BF16) for h in HT]
    preds = np.zeros((t_steps, 2, BL), f32)
    for s in range(t_steps):
        # GRU0
        gh0 = mm_bmajor(stat_tiles(HB[0]), W["w0"])          # (BL, 1536)
        g0 = t128(gh0)                                        # (128,12,4)
        rz0 = 1.0 / (1.0 + np.exp(-(g0[:, :8] + gi0[:, s, :8].astype(f32))))
        n0 = np.tanh(gi0[:, s, 8:].astype(f32) +
                     rz0[:, :4] * (g0[:, 8:] + W["bhhn0"][:, :, None]))
        c0 = n0 + rz0[:, 4:8] * (HT[0] - n0)
        cb0 = c0.astype(BF16)
        cells, cellsb = [c0], [cb0]
        for l in (1, 2):
            stat = stat_tiles(cellsb[-1]) + stat_tiles(HB[l])
            wcat = W[f"w{l}cat"]
            comb_rz = mm_bmajor(stat, wcat[:, :, :1024])
            gin = np.zeros((BL, 512), f32)
            ghn = np.zeros((BL, 512), f32)
            for u in range(4):
                gin += stat[u].astype(f32).T @ wcat[:, u, 1024:].astype(f32)
                ghn += stat[4 + u].astype(f32).T @ \
                    wcat[:, 4 + u, 1024:].astype(f32)
            rzT = t128(comb_rz)
            ginT, ghnT = t128(gin), t128(ghn)
            rz = 1.0 / (1.0 + np.exp(-(rzT + W[f"brz{l}"][:, :, None])))
            nn = np.tanh(ginT + W[f"bihn{l}"][:, :, None] +
                         rz[:, :4] * (ghnT + W[f"bhhn{l}"][:, :, None]))
            c = nn + rz[:, 4:8] * (HT[l] - nn)
            cells.append(c)
            cellsb.append(c.astype(BF16))
        # MLPs -> p (2, BL)
        p = np.zeros((2, BL), f32)
        for l in (0, 1):
            h1 = mm_bmajor(stat_tiles(cellsb[l]), W[f"mw1_{l}"])
            h1 = np.maximum(t128(h1) + W[f"mb1_{l}"][:, :, None], 0.0)
            h1b = h1.astype(BF16)
            h2 = mm_bmajor(stat_tiles(h1b), W[f"mw2_{l}"])
            h2 = np.maximum(t128(h2) + W[f"mb2_{l}"][:, :, None], 0.0)
            h2b = h2.astype(BF16)
            acc = np.zeros(BL, f32)
            for u in range(NU):
                acc += W[f"mw3_{l}"][:, u, 0].astype(f32) @ h2b[:, u].astype(f32)
            p[l] = 1.0 / (1.0 + np.exp(-(acc + W[f"mb3_{l}"])))
        preds[s] = p
        a, b = p[0], p[1]
        E0, E1, E2 = np.exp(1 - a), np.exp(a * (1 - b)), np.exp(a * b)
        S = E0 + E1 + E2
        m0 = (E0 / S).astype(BF16).astype(f32)
        m1 = (E1 / S).astype(BF16).astype(f32)
        m2 = (E2 / S).astype(BF16).astype(f32)
        F0, F1 = np.exp(1 - b), np.exp(b)
        Sf = F0 + F1
        n0_ = (F0 / Sf).astype(BF16).astype(f32)
        n1_ = (F1 / Sf).astype(BF16).astype(f32)
        HT[0] = m0 * cells[0] + m1 * cells[1] + m2 * cells[2]
        HT[1] = n0_ * cells[1] + n1_ * cells[2]
        HT[2] = cells[2]
        HB = [h.astype(BF16) for h in HT]
    # -> (BL, t_steps, 2)
    return preds.transpose(2, 0, 1)


# --------------------------------------------------------------------------
# bass program
# --------------------------------------------------------------------------

_CACHE = {}


def build_program(t_steps=T):
    import concourse.bacc as bacc
    import concourse.bass as bass
    import concourse.tile as tile
    import concourse.mybir as mybir

    dt = mybir.dt
    F32, BF = dt.float32, dt.bfloat16
    AF = mybir.ActivationFunctionType

    nc = bacc.Bacc("TRN2", target_bir_lowering=False, debug=False,
                   num_devices=NCORES)

    dram = {}

    def din(name, shape, dtype):
        dram[name] = nc.dram_tensor(name, list(shape), dtype,
                                    kind="ExternalInput")
        return dram[name]

    din("xT", (128, 6, T * BL), BF)
    din("wihA", (128, 6, NT, 128), BF)
    din("w0", (128, NU, G), BF)
    din("w1cat", (128, 2 * NU, G), BF)
    din("w2cat", (128, 2 * NU, G), BF)
    for l in (0, 1):
        din(f"mw1_{l}", (128, NU, H), BF)
        din(f"mw2_{l}", (128, NU, H), BF)
        din(f"mw3_{l}", (128, NU, 1), BF)
    din("biasA", (128, NT), F32)
    din("bhhn0", (128, NU), F32)
    for l in (1, 2):
        din(f"brz{l}", (128, 8), F32)
        din(f"bihn{l}", (128, NU), F32)
        din(f"bhhn{l}", (128, NU), F32)
    for l in (0, 1):
        din(f"mb1_{l}", (128, NU), F32)
        din(f"mb2_{l}", (128, NU), F32)
    din("mb3", (1, 2), F32)
    pout = nc.dram_tensor("pout", [1, t_steps, 2, BL], F32,
                          kind="ExternalOutput")

    with tile.TileContext(nc) as tc, ExitStack() as ctx:
        pers = ctx.enter_context(tc.tile_pool(name="pers", bufs=1))
        psG = ctx.enter_context(
            tc.tile_pool(name="psG", bufs=3, space="PSUM"))
        psM = ctx.enter_context(
            tc.tile_pool(name="psM", bufs=2, space="PSUM"))
        scr = ctx.enter_context(tc.tile_pool(name="scr", bufs=4))

        def sb_in(name):
            t_ = pers.tile(list(dram[name].shape), dram[name].dtype,
                           tag=name)
            nc.gpsimd.dma_start(t_[:], dram[name][:])
            return t_

        w0 = sb_in("w0")
        wcat = {1: sb_in("w1cat"), 2: sb_in("w2cat")}
        mw = {(w, l): sb_in(f"mw{w}_{l}") for w in (1, 2, 3) for l in (0, 1)}
        biasA = sb_in("biasA")
        bhhn = {0: sb_in("bhhn0"), 1: sb_in("bhhn1"), 2: sb_in("bhhn2")}
        brz = {1: sb_in("brz1"), 2: sb_in("brz2")}
        bihn = {1: sb_in("bihn1"), 2: sb_in("bihn2")}
        mb = {(w, l): sb_in(f"mb{w}_{l}") for w in (1, 2) for l in (0, 1)}
        mb3 = sb_in("mb3")

        gi0 = pers.tile([128, t_steps, NT, BL], BF, tag="gi0", name="gi0")
        nc.gpsimd.memset(gi0[:], 0.0)
        onesb = pers.tile([1, 128], BF, tag="ones", name="ones")
        nc.gpsimd.memset(onesb[:], 1.0)

        # ---- Phase A (two halves so xT staging is 12KB/partition) ----
        wA = pers.tile([128, 6, NT, 128], BF, tag="wihA", name="wihA")
        nc.gpsimd.dma_start(wA[:], dram["wihA"][:])
        half = min(max(512, (t_steps * BL) // 2), t_steps * BL)
        cs = min(512, half)
        for h_ in range((t_steps * BL) // half):
            xT = pers.tile([128, 6, half], BF, tag="xT", name="xT")
            nc.gpsimd.dma_start(
                xT[:], dram["xT"][:, :, half * h_:half * (h_ + 1)])
            for t in range(NT):
                for c in range(half // cs):
                    pa = psG.tile([128, cs], F32, tag="pg", name="pg")
                    for kt in range(6):
                        nc.tensor.matmul(
                            pa[:], wA[:, kt, t, :],
                            xT[:, kt, cs * c:cs * (c + 1)],
                            start=(kt == 0), stop=(kt == 5))
                    s0 = (half * h_ + cs * c) // BL
                    nc.scalar.activation(
                        gi0[:, s0:s0 + cs // BL, t, :],
                        pa[:], AF.Identity, bias=biasA[:, t:t + 1],
                        scale=1.0)

        # ---- state ----
        HT = {0: pers.tile([128, NU, BL], F32, tag="HT0", name="HT0"),
              1: pers.tile([128, NU, BL], F32, tag="HT1", name="HT1")}
        HB = {0: pers.tile([128, NU, BL], BF, tag="HB0", name="HB0"),
              1: pers.tile([128, NU, BL], BF, tag="HB1", name="HB1")}
        for l in (0, 1):
            nc.gpsimd.memset(HT[l][:], 0.0)
            nc.gpsimd.memset(HB[l][:], 0.0)
        ct2z = pers.tile([128, NU, BL], F32, tag="ct2z", name="ct2z")
        cb2z = pers.tile([128, NU, BL], BF, tag="cb2z", name="cb2z")
        nc.gpsimd.memset(ct2z[:], 0.0)
        nc.gpsimd.memset(cb2z[:], 0.0)
        out_acc = pers.tile([1, t_steps, 2, BL], F32, tag="out", name="out")
        nc.gpsimd.memset(out_acc[:], 0.0)

        def transp(dst, src_psum, ng, g0=0):
            # src (32, ngtot, 4, 32) psum B-major -> dst (128, ngtot, 32),
            # groups g0:g0+ng
            for q in range(4):
                nc.vector.transpose(
                    dst[32 * q:32 * (q + 1), g0:g0 + ng, :],
                    src_psum[:, g0:g0 + ng, q, :])

        def gate_mm(psum_t, stats, wmov, u_base, n0_, ng, c_base=0):
            # psum_t: [32, ngtot, 4, 32]; accumulate k-tiles per 512-chunk of
            # wmov (tile) columns starting at n0_; write chunks starting at
            # group 4*c_base of psum_t
            nkk = len(stats)
            for c in range((ng * 128) // 512):
                lo = n0_ + 512 * c
                cc = c_base + c
                for u in range(nkk):
                    nc.tensor.matmul(
                        psum_t[0:BL, 4 * cc:4 * (cc + 1), :, :],
                        stats[u], wmov[:, u_base + u, lo:lo + 512],
                        start=(u == 0), stop=(u == nkk - 1))

        def bcb(col, n=NU):
            # bias tile AP (128, n) -> broadcast (128, n, BL)
            return col.unsqueeze(2).broadcast_to((128, n, BL))

        def bcc(col):
            # coef column AP (128, BL) -> broadcast (128, NU, BL)
            return col.unsqueeze(1).broadcast_to((128, NU, BL))

        # previous-step cell2 (aliased as layer-2 state); ht2 is an AP,
        # cb2 a tile
        ct2_prev, cb2_prev = ct2z[:], cb2z

        for s in range(t_steps):
            CTcat = scr.tile([128, NU, BL, L], F32, tag="CTcat", name="CTcat")
            CT = [CTcat[:, :, :, l] for l in range(L)]
            CB = [scr.tile([128, NU, BL], BF, tag=f"CB{l}", name=f"CB{l}")
                  for l in range(L)]
            ht2, hb2 = ct2_prev, cb2_prev

            # ---------- GRU0 ----------
            AL = mybir.AluOpType

            def gru_tail(l, rz, nT, ht_ap, gi_n=None):
                # rz holds y = tanh((gates+bias)/2); sigmoid = 0.5y + 0.5
                rf = scr.tile([128, NU, BL], F32, tag="rf", name="rf")
                nc.vector.tensor_scalar(rf[:], rz[:, 0:4, :], 0.5, 0.5,
                                        AL.mult, AL.add)       # r
                zt = scr.tile([128, NU, BL], F32, tag="zt", name="zt")
                nc.gpsimd.tensor_scalar(zt[:], rz[:, 4:8, :], 0.5, 0.5,
                                        AL.mult, AL.add)       # z
                omz = scr.tile([128, NU, BL], F32, tag="omz", name="omz")
                nc.vector.tensor_scalar(omz[:], rz[:, 4:8, :], -0.5, 0.5,
                                        AL.mult, AL.add)       # 1-z
                t5 = scr.tile([128, NU, BL], F32, tag="t5", name="t5")
                nc.gpsimd.tensor_mul(t5[:], zt[:], ht_ap)      # z*h
                t1 = scr.tile([128, NU, BL], F32, tag="t1", name="t1")
                if l == 0:
                    nc.vector.tensor_add(t1[:], nT[:, 0:4, 0:BL],
                                         bcb(bhhn[0][:]))
                    nc.vector.tensor_mul(t1[:], rf[:], t1[:])
                    nc.vector.tensor_add(t1[:], t1[:], gi0[:, s, 8:12, :])
                else:
                    nc.vector.tensor_add(t1[:], nT[:, 4:8, 0:BL],
                                         bcb(bhhn[l][:]))
                    nc.vector.tensor_mul(t1[:], rf[:], t1[:])
                    t2 = scr.tile([128, NU, BL], F32, tag="t2", name="t2")
                    nc.gpsimd.tensor_add(t2[:], nT[:, 0:4, 0:BL],
                                         bcb(bihn[l][:]))
                    nc.vector.tensor_add(t1[:], t1[:], t2[:])
                nn_ = scr.tile([128, NU, BL], F32, tag="nn", name="nn")
                nc.scalar.activation(nn_[:], t1[:], AF.Tanh)
                nc.vector.tensor_mul(nn_[:], omz[:], nn_[:])   # (1-z)*n
                nc.gpsimd.tensor_add(CT[l], nn_[:], t5[:])
                nc.scalar.copy(CB[l][:], CT[l])

            stats0 = [HB[0][:, u, :] for u in range(NU)]
            pgA = psG.tile([32, 8, 4, 32], F32, tag="pg", name="pg")
            gate_mm(pgA, stats0, w0, 0, 0, 8)
            pgB = psG.tile([32, 8, 4, 32], F32, tag="pg", name="pg")
            gate_mm(pgB, stats0, w0, 0, 1024, 4)
            rzT = scr.tile([128, 8, 32], F32, tag="rzT", name="rzT")
            transp(rzT, pgA, 4, 0)
            rz = scr.tile([128, 8, BL], F32, tag="rz", name="rz")
            nc.vector.tensor_add(rz[:, 0:4, :], rzT[:, 0:4, 0:BL],
                                 gi0[:, s, 0:4, :])
            transp(rzT, pgA, 4, 4)
            nc.vector.tensor_add(rz[:, 4:8, :], rzT[:, 4:8, 0:BL],
                                 gi0[:, s, 4:8, :])
            nc.scalar.activation(rz[:], rz[:], AF.Tanh, scale=0.5)
            nT = scr.tile([128, 8, 32], F32, tag="nT", name="nT")
            transp(nT, pgB, 4)
            gru_tail(0, rz, nT, HT[0][:])

            # ---------- GRU1 ----------
            def gru_l(l, cbprev, hbl):
                # issue k-tiles whose stationary is already available (the
                # recurrent h state) first, then the cell-dependent ones
                w = wcat[l]
                pa = psG.tile([32, 8, 4, 32], F32, tag="pg", name="pg")
                pb = psG.tile([32, 8, 4, 32], F32, tag="pg", name="pg")
                for c in (0, 1):                      # rz chunks, h k-tiles
                    for j, u in enumerate(range(4, 8)):
                        nc.tensor.matmul(
                            pa[0:BL, 4 * c:4 * (c + 1), :, :],
                            hbl[:, u - 4, :], w[:, u, 512 * c:512 * (c + 1)],
                            start=(j == 0), stop=False)
                for j, u in enumerate(range(4, 8)):   # ghn chunk (h only)
                    nc.tensor.matmul(
                        pb[0:BL, 4:8, :, :],
                        hbl[:, u - 4, :], w[:, u, 1024:1536],
                        start=(j == 0), stop=(j == 3))
                for c in (0, 1):                      # rz chunks, c k-tiles
                    for j, u in enumerate(range(0, 4)):
                        nc.tensor.matmul(
                            pa[0:BL, 4 * c:4 * (c + 1), :, :],
                            cbprev[:, u, :], w[:, u, 512 * c:512 * (c + 1)],
                            start=False, stop=(j == 3))
                for j, u in enumerate(range(0, 4)):   # gin chunk (c only)
                    nc.tensor.matmul(
                        pb[0:BL, 0:4, :, :],
                        cbprev[:, u, :], w[:, u, 1024:1536],
                        start=(j == 0), stop=(j == 3))
                return pa, pb

            def gru_math(l, pa, pb, ht_ap):
                rzTl = scr.tile([128, 8, 32], F32, tag="rzT", name="rzT")
                transp(rzTl, pa, 4, 0)
                rzl = scr.tile([128, 8, BL], F32, tag="rz", name="rz")
                nc.vector.tensor_add(rzl[:, 0:4, :], rzTl[:, 0:4, 0:BL],
                                     bcb(brz[l][:, 0:4]))
                transp(rzTl, pa, 4, 4)
                nc.vector.tensor_add(rzl[:, 4:8, :], rzTl[:, 4:8, 0:BL],
                                     bcb(brz[l][:, 4:8]))
                nc.scalar.activation(rzl[:], rzl[:], AF.Tanh, scale=0.5)
                nTl = scr.tile([128, 8, 32], F32, tag="nT", name="nT")
                transp(nTl, pb, 8)
                gru_tail(l, rzl, nTl, ht_ap)

            pa1, pb1 = gru_l(1, CB[0], HB[1])
            # MLP0 w1 matmuls can run under GRU1 math
            pm0 = psM.tile([32, 4, 4, 32], F32, tag="pm", name="pm")
            for u in range(NU):
                nc.tensor.matmul(pm0[0:BL, :, :, :], CB[0][:, u, :],
                                 mw[(1, 0)][:, u, :], start=(u == 0),
                                 stop=(u == 3))
            gru_math(1, pa1, pb1, HT[1][:])

            pa2, pb2 = gru_l(2, CB[1], hb2)

            # MLP0 w1 math + w2 matmuls under GRU2 math
            def mlp_stage(pm, wl, l):
                mt = scr.tile([128, 4, 32], F32, tag="mt", name="mt")
                transp(mt, pm, 4)
                hb_ = scr.tile([128, NU, BL], BF, tag=f"hb{wl}_{l}",
                               name=f"hb{wl}_{l}")
                nc.vector.tensor_add(mt[:, :, 0:BL], mt[:, :, 0:BL],
                                     bcb(mb[(wl, l)][:]))
                nc.gpsimd.tensor_relu(hb_[:], mt[:, :, 0:BL])
                return hb_

            h1b0 = mlp_stage(pm0, 1, 0)
            pm0b = psM.tile([32, 4, 4, 32], F32, tag="pm", name="pm")
            for u in range(NU):
                nc.tensor.matmul(pm0b[0:BL, :, :, :], h1b0[:, u, :],
                                 mw[(2, 0)][:, u, :], start=(u == 0),
                                 stop=(u == 3))

            # MLP1 w1; the whole P path is emitted before GRU2's math so
            # its transposes don't queue behind GRU2's in the DVE FIFO --
            # CT2 isn't needed until the mixing apply
            pm1 = psM.tile([32, 4, 4, 32], F32, tag="pm", name="pm")
            for u in range(NU):
                nc.tensor.matmul(pm1[0:BL, :, :, :], CB[1][:, u, :],
                                 mw[(1, 1)][:, u, :], start=(u == 0),
                                 stop=(u == 3))

            P = scr.tile([1, 2, BL], F32, tag="P", name="P")

            h2b0 = mlp_stage(pm0b, 2, 0)
            p30 = psM.tile([1, BL], F32, tag="pm", name="p30")
            for u in range(NU):
                nc.tensor.matmul(p30[:], mw[(3, 0)][:, u, :],
                                 h2b0[:, u, :], start=(u == 0), stop=(u == 3))
            nc.scalar.activation(P[:, 0, :], p30[:], AF.Tanh,
                                 bias=mb3[:, 0:1], scale=0.5)
            nc.vector.tensor_scalar(P[:, 0, :], P[:, 0, :], 0.5, 0.5,
                                    AL.mult, AL.add)
            X = scr.tile([1, 5, BL], F32, tag="X", name="X")
            nc.scalar.activation(X[:, 0, :], P[:, 0, :], AF.Identity,
                                 bias=1.0, scale=-1.0)          # 1-a (early)

            h1b1 = mlp_stage(pm1, 1, 1)
            pm1b = psM.tile([32, 4, 4, 32], F32, tag="pm", name="pm")
            for u in range(NU):
                nc.tensor.matmul(pm1b[0:BL, :, :, :], h1b1[:, u, :],
                                 mw[(2, 1)][:, u, :], start=(u == 0),
                                 stop=(u == 3))
            h2b1 = mlp_stage(pm1b, 2, 1)
            p31 = psM.tile([1, BL], F32, tag="pm", name="p31")
            for u in range(NU):
                nc.tensor.matmul(p31[:], mw[(3, 1)][:, u, :],
                                 h2b1[:, u, :], start=(u == 0), stop=(u == 3))
            nc.scalar.activation(P[:, 1, :], p31[:], AF.Tanh,
                                 bias=mb3[:, 1:2], scale=0.5)
            nc.vector.tensor_scalar(P[:, 1, :], P[:, 1, :], 0.5, 0.5,
                                    AL.mult, AL.add)

            nc.gpsimd.tensor_copy(out_acc[:, s], P[:])

            # ---------- mixing coefficients on [1, BL] tiles ----------
            a = P[:, 0, :]
            b_ = P[:, 1, :]
            nc.vector.tensor_mul(X[:, 2, :], a, b_)              # ab
            nc.vector.tensor_sub(X[:, 1, :], a, X[:, 2, :])      # a-ab
            nc.scalar.activation(X[:, 3, :], b_, AF.Identity,
                                 bias=1.0, scale=-1.0)          # 1-b
            nc.vector.tensor_copy(X[:, 4, :], b_)                # b
            E = scr.tile([1, 5, BL], F32, tag="E", name="E")
            nc.scalar.activation(E[:], X[:], AF.Exp)
            Ssum = scr.tile([1, 2, BL], F32, tag="Ssum", name="Ssum")
            nc.vector.tensor_add(Ssum[:, 0, :], E[:, 0, :], E[:, 1, :])
            nc.vector.tensor_add(Ssum[:, 0, :], Ssum[:, 0, :], E[:, 2, :])
            nc.vector.tensor_add(Ssum[:, 1, :], E[:, 3, :], E[:, 4, :])
            nc.vector.reciprocal(Ssum[:], Ssum[:])
            Mb_ = scr.tile([1, 5, BL], BF, tag="Mb", name="Mb")
            nc.vector.tensor_mul(
                Mb_[:, 0:3, :], E[:, 0:3, :],
                Ssum[:, 0, :].unsqueeze(1).broadcast_to((1, 3, BL)))
            nc.vector.tensor_mul(
                Mb_[:, 3:5, :], E[:, 3:5, :],
                Ssum[:, 1, :].unsqueeze(1).broadcast_to((1, 2, BL)))
            pbx = psM.tile([128, 5, BL], F32, tag="pm", name="pbx")
            nc.tensor.matmul(pbx[:], onesb[:], Mb_[:],
                             start=True, stop=True)
            coef = scr.tile([128, 5, BL], F32, tag="coef", name="coef")
            nc.scalar.copy(coef[:], pbx[:])

            # ---------- mixing apply, CT0/CT1 terms ----------
            # hoisted before GRU2's math: they need only coef, so after CT2
            # lands just two ops separate it from the HB0/HB1 handoff
            ta = scr.tile([128, NU, BL], F32, tag="ta", name="ta")
            tb = scr.tile([128, NU, BL], F32, tag="tb", name="tb")
            tc_ = scr.tile([128, NU, BL], F32, tag="tc", name="tc")
            td_ = scr.tile([128, NU, BL], F32, tag="td", name="td")
            nc.vector.tensor_mul(ta[:], CT[0], bcc(pbx[:, 0, :]))
            nc.gpsimd.tensor_mul(tb[:], CT[1], bcc(coef[:, 1, :]))
            nc.vector.tensor_add(ta[:], ta[:], tb[:])
            nc.gpsimd.tensor_mul(tc_[:], CT[1], bcc(coef[:, 3, :]))

            gru_math(2, pa2, pb2, ht2)

            # ---------- mixing apply, CT2 terms ----------
            nc.gpsimd.tensor_mul(tb[:], CT[2], bcc(coef[:, 2, :]))
            nc.vector.tensor_add(HB[0][:], ta[:], tb[:])
            nc.gpsimd.tensor_add(HT[0][:], ta[:], tb[:])
            nc.vector.tensor_mul(td_[:], CT[2], bcc(pbx[:, 4, :]))
            nc.vector.tensor_add(HB[1][:], tc_[:], td_[:])
            nc.gpsimd.tensor_add(HT[1][:], tc_[:], td_[:])
            ct2_prev, cb2_prev = CT[2], CB[2]

        nc.gpsimd.dma_start(pout[:], out_acc[:])

    nc.finalize()
    return nc


def _kernel_numpy(np_inputs, t_steps):
    W = prep_host(np_inputs)
    outs = [emulate_core(W, np_inputs["x"][BL * c:BL * (c + 1)].astype(
        np.float32), t_steps=t_steps) for c in range(NCORES)]
    return np.concatenate(outs, axis=0).astype(np.float32)


def kernel(_t_steps=T, **inputs):
    np_inputs = {k: np.asarray(v) for k, v in inputs.items()}
    if os.environ.get("HRNN_NUMPY"):
        return _kernel_numpy(np_inputs, _t_steps)
    return _kernel_bass(np_inputs, _t_steps)


def _kernel_bass(np_inputs, _t_steps):
    W = prep_host(np_inputs)
    t_steps = _t_steps
    key = t_steps
    if key not in _CACHE:
        _CACHE[key] = build_program(t_steps)
    nc = _CACHE[key]
    shared = {k: v for k, v in W.items()
              if k not in ("mb3_0", "mb3_1")}
    # P path computes sigmoid as 0.5*(1+tanh((x+b)/2)) -> bias input is b/2
    mb3 = np.array([[0.5 * W["mb3_0"], 0.5 * W["mb3_1"]]], np.float32)
    x = np_inputs["x"].astype(np.float32)
    in_maps = []
    for c in range(NCORES):
        m = dict(shared)
        m["mb3"] = mb3
        m["xT"] = prep_x(x[BL * c:BL * (c + 1)])
        in_maps.append(m)
    from concourse.bass_utils import run_bass_kernel_spmd
    res = run_bass_kernel_spmd(nc, in_maps, core_ids=list(range(NCORES)))
    outs = []
    for c in range(NCORES):
        p = res.results[c]["pout"].reshape(t_steps, 2, BL)
        outs.append(p.transpose(2, 0, 1))
    return np.concatenate(outs, axis=0).astype(np.float32)


if __name__ == "__main__":
    pass


# revision 31
# speedup vs baseline: 1.0007x; 1.0007x over previous
"""HRNN (L=3, H=512) Trainium2 Bass kernel.

Strategy: data-parallel over batch (32 -> 4 rows/core on 8 cores).
Per core everything lives in a permuted H-major ("T128") SBUF layout:
slot (p=32q+i, group t) holds gate/hidden index 32*(4t+q)+i, batch on a
4-wide free dim.  Gate matmuls run batch-major (stationary = activation
k-tiles [128,4], moving = resident bf16 weights, PSUM B-major) and are
rotated into T128 by DVE 32x32 stream-transposes; weight rows/cols are
pre-permuted on the host so no other shuffles are needed.  x @ Wih0^T for
all timesteps is computed on-device in a prologue (weights-stationary, so
it lands H-major directly) and kept in SBUF as bf16.

v2: gate PSUM split into rz/n halves (ping-pong with DVE transposes),
elementwise chains spread across DVE/Pool/ACT, mixing coefficients
computed on [1,BL] tiles then broadcast once via PE, HT2/HB2 alias the
previous step's CT2/CB2, MLP matmuls interleave under GRU math.
"""

import os
import sys
from contextlib import ExitStack

import numpy as np

sys.path.insert(0, "/opt/trn_rl_repo")

import ml_dtypes  # noqa: E402

BF16 = ml_dtypes.bfloat16

L, H, B, T, D, NCORES = 3, 512, 32, 512, 768, 8
BL = B // NCORES          # 4 batch rows per core
G = 3 * H                 # 1536 gates
NT = G // 128             # 12 T128 groups
NU = H // 128             # 4 h k-tiles


def _gate_perm():
    # perm[t*128 + p] = gate index at T128 slot (p, t)
    p = np.arange(128)
    q, i = p // 32, p % 32
    out = np.zeros(G, np.int64)
    for t in range(NT):
        out[t * 128: (t + 1) * 128] = 32 * (4 * t + q) + i
    return out


def _h_perm():
    p = np.arange(128)
    q, i = p // 32, p % 32
    out = np.zeros(H, np.int64)
    for u in range(NU):
        out[u * 128: (u + 1) * 128] = 32 * (4 * u + q) + i
    return out


GPERM = _gate_perm()
HPERM = _h_perm()


def _mov(WT_perm, nk, n):
    # (nk*128, n) row-permuted W^T -> (128, nk, n) moving-weight layout
    return np.ascontiguousarray(
        WT_perm.reshape(nk, 128, n).transpose(1, 0, 2)).astype(BF16)


def _t128_vec(v):
    # (G,) or (H,) vector -> (128, ngroups) T128 grid
    perm = GPERM if v.shape[0] == G else HPERM
    n = v.shape[0] // 128
    return np.ascontiguousarray(
        v[perm].reshape(n, 128).T).astype(np.float32)


def prep_host(inp):
    """Build the per-core / shared device tensors from the raw inputs."""
    f32 = np.float32
    W = {}
    # Phase A stationary: Wih0 cols gate-permuted, lhsT layout
    WihA = inp["Wih0"][GPERM].T.astype(f32)            # (768, 1536)
    W["wihA"] = np.ascontiguousarray(
        WihA.reshape(6, 128, NT, 128).transpose(1, 0, 2, 3)).astype(BF16)
    # GRU0 moving: Whh0^T rows h-permuted
    W["w0"] = _mov(inp["Whh0"].T[HPERM], NU, G)
    # GRU1/2 moving: [Wih_l^T; Whh_l^T] rows h-permuted per half
    for l in (1, 2):
        cat = np.concatenate([inp["Wih"][l - 1].T[HPERM],
                              inp["Whh"][l - 1].T[HPERM]], axis=0)
        W[f"w{l}cat"] = _mov(cat, 2 * NU, G)
    # MLP moving weights (batch-major) + W3 stationary
    for l in (0, 1):
        W[f"mw1_{l}"] = _mov(inp["mW1"][l].T[HPERM], NU, H)
        W[f"mw2_{l}"] = _mov(inp["mW2"][l].T[HPERM], NU, H)
        w3 = inp["mW3"][l][0][HPERM].astype(f32)       # (512,)
        W[f"mw3_{l}"] = np.ascontiguousarray(
            w3.reshape(NU, 128).T[:, :, None]).astype(BF16)  # (128,4,1)
    # biases
    bA = inp["bih0"].astype(f32).copy()
    bA[:1024] += inp["bhh0"][:1024].astype(f32)
    W["biasA"] = _t128_vec(bA)                          # (128, 12)
    W["bhhn0"] = _t128_vec(inp["bhh0"][1024:].astype(f32))   # (128, 4)
    for l in (1, 2):
        brz = (inp["bih"][l - 1][:1024] + inp["bhh"][l - 1][:1024]).astype(f32)
        W[f"brz{l}"] = _t128_vec(np.concatenate([brz, np.zeros(512, f32)]))[:, :8]
        W[f"bihn{l}"] = _t128_vec(inp["bih"][l - 1][1024:].astype(f32))
        W[f"bhhn{l}"] = _t128_vec(inp["bhh"][l - 1][1024:].astype(f32))
    for l in (0, 1):
        W[f"mb1_{l}"] = _t128_vec(inp["mb1"][l].astype(f32))  # (128,4)
        W[f"mb2_{l}"] = _t128_vec(inp["mb2"][l].astype(f32))
        W[f"mb3_{l}"] = np.float32(inp["mb3"][l][0])
    return W


def prep_x(x_core):
    # (BL, T, D) -> (128, 6, T*BL) bf16, free order (step, batch)
    xt = x_core.transpose(2, 1, 0).reshape(D, T * BL)
    return np.ascontiguousarray(
        xt.reshape(6, 128, T * BL).transpose(1, 0, 2)).astype(BF16)


# --------------------------------------------------------------------------
# numpy emulator of the device program (layout validation)
# --------------------------------------------------------------------------

def emulate_core(W, x_core, t_steps=T):
    f32 = np.float32

    def mm_bmajor(stat_list, mov):
        # stat_list: list of (128, BL) k-tiles (bf16); mov: (128, nk, N)
        acc = np.zeros((BL, mov.shape[2]), f32)
        for u, s in enumerate(stat_list):
            acc += s.astype(f32).T @ mov[:, u, :].astype(f32)
        return acc

    def t128(bmaj):
        # (BL, N) plain gate order -> (128, N//128, 4) compact T128
        N = bmaj.shape[1]
        return np.ascontiguousarray(
            bmaj.reshape(BL, N // 128, 4, 32).transpose(2, 3, 1, 0)
            .reshape(128, N // 128, BL)).astype(f32)

    def stat_tiles(ht_b):  # (128, nu, BL) bf16 -> list of (128, BL)
        return [ht_b[:, u, :] for u in range(ht_b.shape[1])]

    xT = prep_x(x_core)  # (128, 6, T*BL) bf16
    # Phase A: gi0 (128, T, 12, 4) bf16
    gi0 = np.zeros((128, t_steps, NT, BL), BF16)
    for t in range(NT):
        acc = np.zeros((128, t_steps * BL), f32)
        for kt in range(6):
            acc += W["wihA"][:, kt, t, :].astype(f32).T @ \
                xT[:, kt, :t_steps * BL].astype(f32)
        acc += W["biasA"][:, t][:, None]
        gi0[:, :, t, :] = acc.reshape(128, t_steps, BL).astype(BF16)

    HT = [np.zeros((128, NU, BL), f32) for _ in range(L)]
    HB = [h.astype(BF16) for h in HT]
    preds = np.zeros((t_steps, 2, BL), f32)
    for s in range(t_steps):
        # GRU0
        gh0 = mm_bmajor(stat_tiles(HB[0]), W["w0"])          # (BL, 1536)
        g0 = t128(gh0)                                        # (128,12,4)
        rz0 = 1.0 / (1.0 + np.exp(-(g0[:, :8] + gi0[:, s, :8].astype(f32))))
        n0 = np.tanh(gi0[:, s, 8:].astype(f32) +
                     rz0[:, :4] * (g0[:, 8:] + W["bhhn0"][:, :, None]))
        c0 = n0 + rz0[:, 4:8] * (HT[0] - n0)
        cb0 = c0.astype(BF16)
        cells, cellsb = [c0], [cb0]
        for l in (1, 2):
            stat = stat_tiles(cellsb[-1]) + stat_tiles(HB[l])
            wcat = W[f"w{l}cat"]
            comb_rz = mm_bmajor(stat, wcat[:, :, :1024])
            gin = np.zeros((BL, 512), f32)
            ghn = np.zeros((BL, 512), f32)
            for u in range(4):
                gin += stat[u].astype(f32).T @ wcat[:, u, 1024:].astype(f32)
                ghn += stat[4 + u].astype(f32).T @ \
                    wcat[:, 4 + u, 1024:].astype(f32)
            rzT = t128(comb_rz)
            ginT, ghnT = t128(gin), t128(ghn)
            rz = 1.0 / (1.0 + np.exp(-(rzT + W[f"brz{l}"][:, :, None])))
            nn = np.tanh(ginT + W[f"bihn{l}"][:, :, None] +
                         rz[:, :4] * (ghnT + W[f"bhhn{l}"][:, :, None]))
            c = nn + rz[:, 4:8] * (HT[l] - nn)
            cells.append(c)
            cellsb.append(c.astype(BF16))
        # MLPs -> p (2, BL)
        p = np.zeros((2, BL), f32)
        for l in (0, 1):
            h1 = mm_bmajor(stat_tiles(cellsb[l]), W[f"mw1_{l}"])
            h1 = np.maximum(t128(h1) + W[f"mb1_{l}"][:, :, None], 0.0)
            h1b = h1.astype(BF16)
            h2 = mm_bmajor(stat_tiles(h1b), W[f"mw2_{l}"])
            h2 = np.maximum(t128(h2) + W[f"mb2_{l}"][:, :, None], 0.0)
            h2b = h2.astype(BF16)
            acc = np.zeros(BL, f32)
            for u in range(NU):
                acc += W[f"mw3_{l}"][:, u, 0].astype(f32) @ h2b[:, u].astype(f32)
            p[l] = 1.0 / (1.0 + np.exp(-(acc + W[f"mb3_{l}"])))
        preds[s] = p
        a, b = p[0], p[1]
        E0, E1, E2 = np.exp(1 - a), np.exp(a * (1 - b)), np.exp(a * b)
        S = E0 + E1 + E2
        m0 = (E0 / S).astype(BF16).astype(f32)
        m1 = (E1 / S).astype(BF16).astype(f32)
        m2 = (E2 / S).astype(BF16).astype(f32)
        F0, F1 = np.exp(1 - b), np.exp(b)
        Sf = F0 + F1
        n0_ = (F0 / Sf).astype(BF16).astype(f32)
        n1_ = (F1 / Sf).astype(BF16).astype(f32)
        HT[0] = m0 * cells[0] + m1 * cells[1] + m2 * cells[2]
        HT[1] = n0_ * cells[1] + n1_ * cells[2]
        HT[2] = cells[2]
        HB = [h.astype(BF16) for h in HT]
    # -> (BL, t_steps, 2)
    return preds.transpose(2, 0, 1)


# --------------------------------------------------------------------------
# bass program
# --------------------------------------------------------------------------

_CACHE = {}


def build_program(t_steps=T):
    import concourse.bacc as bacc
    import concourse.bass as bass
    import concourse.tile as tile
    import concourse.mybir as mybir

    dt = mybir.dt
    F32, BF = dt.float32, dt.bfloat16
    AF = mybir.ActivationFunctionType

    nc = bacc.Bacc("TRN2", target_bir_lowering=False, debug=False,
                   num_devices=NCORES)

    dram = {}

    def din(name, shape, dtype):
        dram[name] = nc.dram_tensor(name, list(shape), dtype,
                                    kind="ExternalInput")
        return dram[name]

    din("xT", (128, 6, T * BL), BF)
    din("wihA", (128, 6, NT, 128), BF)
    din("w0", (128, NU, G), BF)
    din("w1cat", (128, 2 * NU, G), BF)
    din("w2cat", (128, 2 * NU, G), BF)
    for l in (0, 1):
        din(f"mw1_{l}", (128, NU, H), BF)
        din(f"mw2_{l}", (128, NU, H), BF)
        din(f"mw3_{l}", (128, NU, 1), BF)
    din("biasA", (128, NT), F32)
    din("bhhn0", (128, NU), F32)
    for l in (1, 2):
        din(f"brz{l}", (128, 8), F32)
        din(f"bihn{l}", (128, NU), F32)
        din(f"bhhn{l}", (128, NU), F32)
    for l in (0, 1):
        din(f"mb1_{l}", (128, NU), F32)
        din(f"mb2_{l}", (128, NU), F32)
    din("mb3", (1, 2), F32)
    pout = nc.dram_tensor("pout", [1, t_steps, 2, BL], F32,
                          kind="ExternalOutput")

    with tile.TileContext(nc) as tc, ExitStack() as ctx:
        pers = ctx.enter_context(tc.tile_pool(name="pers", bufs=1))
        psG = ctx.enter_context(
            tc.tile_pool(name="psG", bufs=3, space="PSUM"))
        psM = ctx.enter_context(
            tc.tile_pool(name="psM", bufs=2, space="PSUM"))
        scr = ctx.enter_context(tc.tile_pool(name="scr", bufs=4))

        def sb_in(name):
            t_ = pers.tile(list(dram[name].shape), dram[name].dtype,
                           tag=name)
            nc.gpsimd.dma_start(t_[:], dram[name][:])
            return t_

        w0 = sb_in("w0")
        wcat = {1: sb_in("w1cat"), 2: sb_in("w2cat")}
        mw = {(w, l): sb_in(f"mw{w}_{l}") for w in (1, 2, 3) for l in (0, 1)}
        biasA = sb_in("biasA")
        bhhn = {0: sb_in("bhhn0"), 1: sb_in("bhhn1"), 2: sb_in("bhhn2")}
        brz = {1: sb_in("brz1"), 2: sb_in("brz2")}
        bihn = {1: sb_in("bihn1"), 2: sb_in("bihn2")}
        mb = {(w, l): sb_in(f"mb{w}_{l}") for w in (1, 2) for l in (0, 1)}
        mb3 = sb_in("mb3")

        gi0 = pers.tile([128, t_steps, NT, BL], BF, tag="gi0", name="gi0")
        nc.gpsimd.memset(gi0[:], 0.0)
        onesb = pers.tile([1, 128], BF, tag="ones", name="ones")
        nc.gpsimd.memset(onesb[:], 1.0)

        # ---- Phase A (two halves so xT staging is 12KB/partition) ----
        wA = pers.tile([128, 6, NT, 128], BF, tag="wihA", name="wihA")
        nc.gpsimd.dma_start(wA[:], dram["wihA"][:])
        half = min(max(512, (t_steps * BL) // 2), t_steps * BL)
        cs = min(512, half)
        for h_ in range((t_steps * BL) // half):
            xT = pers.tile([128, 6, half], BF, tag="xT", name="xT")
            nc.gpsimd.dma_start(
                xT[:], dram["xT"][:, :, half * h_:half * (h_ + 1)])
            for t in range(NT):
                for c in range(half // cs):
                    pa = psG.tile([128, cs], F32, tag="pg", name="pg")
                    for kt in range(6):
                        nc.tensor.matmul(
                            pa[:], wA[:, kt, t, :],
                            xT[:, kt, cs * c:cs * (c + 1)],
                            start=(kt == 0), stop=(kt == 5))
                    s0 = (half * h_ + cs * c) // BL
                    nc.scalar.activation(
                        gi0[:, s0:s0 + cs // BL, t, :],
                        pa[:], AF.Identity, bias=biasA[:, t:t + 1],
                        scale=1.0)

        # ---- state ----
        HT = {0: pers.tile([128, NU, BL], F32, tag="HT0", name="HT0"),
              1: pers.tile([128, NU, BL], F32, tag="HT1", name="HT1")}
        HB = {0: pers.tile([128, NU, BL], BF, tag="HB0", name="HB0"),
              1: pers.tile([128, NU, BL], BF, tag="HB1", name="HB1")}
        for l in (0, 1):
            nc.gpsimd.memset(HT[l][:], 0.0)
            nc.gpsimd.memset(HB[l][:], 0.0)
        ct2z = pers.tile([128, NU, BL], F32, tag="ct2z", name="ct2z")
        cb2z = pers.tile([128, NU, BL], BF, tag="cb2z", name="cb2z")
        nc.gpsimd.memset(ct2z[:], 0.0)
        nc.gpsimd.memset(cb2z[:], 0.0)
        out_acc = pers.tile([1, t_steps, 2, BL], F32, tag="out", name="out")
        nc.gpsimd.memset(out_acc[:], 0.0)

        def transp(dst, src_psum, ng, g0=0):
            # src (32, ngtot, 4, 32) psum B-major -> dst (128, ngtot, 32),
            # groups g0:g0+ng
            for q in range(4):
                nc.vector.transpose(
                    dst[32 * q:32 * (q + 1), g0:g0 + ng, :],
                    src_psum[:, g0:g0 + ng, q, :])

        def gate_mm(psum_t, stats, wmov, u_base, n0_, ng, c_base=0):
            # psum_t: [32, ngtot, 4, 32]; accumulate k-tiles per 512-chunk of
            # wmov (tile) columns starting at n0_; write chunks starting at
            # group 4*c_base of psum_t
            nkk = len(stats)
            for c in range((ng * 128) // 512):
                lo = n0_ + 512 * c
                cc = c_base + c
                for u in range(nkk):
                    nc.tensor.matmul(
                        psum_t[0:BL, 4 * cc:4 * (cc + 1), :, :],
                        stats[u], wmov[:, u_base + u, lo:lo + 512],
                        start=(u == 0), stop=(u == nkk - 1))

        def bcb(col, n=NU):
            # bias tile AP (128, n) -> broadcast (128, n, BL)
            return col.unsqueeze(2).broadcast_to((128, n, BL))

        def bcc(col):
            # coef column AP (128, BL) -> broadcast (128, NU, BL)
            return col.unsqueeze(1).broadcast_to((128, NU, BL))

        # previous-step cell2 (aliased as layer-2 state); ht2 is an AP,
        # cb2 a tile
        ct2_prev, cb2_prev = ct2z[:], cb2z

        for s in range(t_steps):
            CTcat = scr.tile([128, NU, BL, L], F32, tag="CTcat", name="CTcat")
            CT = [CTcat[:, :, :, l] for l in range(L)]
            CB = [scr.tile([128, NU, BL], BF, tag=f"CB{l}", name=f"CB{l}")
                  for l in range(L)]
            ht2, hb2 = ct2_prev, cb2_prev

            # ---------- GRU0 ----------
            AL = mybir.AluOpType

            def gru_tail(l, rz, nT, ht_ap, gi_n=None):
                # rz holds y = tanh((gates+bias)/2); sigmoid = 0.5y + 0.5
                rf = scr.tile([128, NU, BL], F32, tag="rf", name="rf")
                nc.vector.tensor_scalar(rf[:], rz[:, 0:4, :], 0.5, 0.5,
                                        AL.mult, AL.add)       # r
                zt = scr.tile([128, NU, BL], F32, tag="zt", name="zt")
                nc.gpsimd.tensor_scalar(zt[:], rz[:, 4:8, :], 0.5, 0.5,
                                        AL.mult, AL.add)       # z
                omz = scr.tile([128, NU, BL], F32, tag="omz", name="omz")
                nc.vector.tensor_scalar(omz[:], rz[:, 4:8, :], -0.5, 0.5,
                                        AL.mult, AL.add)       # 1-z
                t5 = scr.tile([128, NU, BL], F32, tag="t5", name="t5")
                nc.gpsimd.tensor_mul(t5[:], zt[:], ht_ap)      # z*h
                t1 = scr.tile([128, NU, BL], F32, tag="t1", name="t1")
                if l == 0:
                    nc.vector.tensor_add(t1[:], nT[:, 0:4, 0:BL],
                                         bcb(bhhn[0][:]))
                    nc.vector.tensor_mul(t1[:], rf[:], t1[:])
                    nc.vector.tensor_add(t1[:], t1[:], gi0[:, s, 8:12, :])
                else:
                    nc.vector.tensor_add(t1[:], nT[:, 4:8, 0:BL],
                                         bcb(bhhn[l][:]))
                    nc.vector.tensor_mul(t1[:], rf[:], t1[:])
                    t2 = scr.tile([128, NU, BL], F32, tag="t2", name="t2")
                    nc.gpsimd.tensor_add(t2[:], nT[:, 0:4, 0:BL],
                                         bcb(bihn[l][:]))
                    nc.vector.tensor_add(t1[:], t1[:], t2[:])
                nn_ = scr.tile([128, NU, BL], F32, tag="nn", name="nn")
                nc.scalar.activation(nn_[:], t1[:], AF.Tanh)
                nc.vector.tensor_mul(nn_[:], omz[:], nn_[:])   # (1-z)*n
                nc.gpsimd.tensor_add(CT[l], nn_[:], t5[:])
                nc.scalar.copy(CB[l][:], CT[l])

            stats0 = [HB[0][:, u, :] for u in range(NU)]
            pgA = psG.tile([32, 8, 4, 32], F32, tag="pg", name="pg")
            gate_mm(pgA, stats0, w0, 0, 0, 8)
            pgB = psG.tile([32, 8, 4, 32], F32, tag="pg", name="pg")
            gate_mm(pgB, stats0, w0, 0, 1024, 4)
            rzT = scr.tile([128, 8, 32], F32, tag="rzT", name="rzT")
            transp(rzT, pgA, 4, 0)
            rz = scr.tile([128, 8, BL], F32, tag="rz", name="rz")
            nc.vector.tensor_add(rz[:, 0:4, :], rzT[:, 0:4, 0:BL],
                                 gi0[:, s, 0:4, :])
            transp(rzT, pgA, 4, 4)
            nc.vector.tensor_add(rz[:, 4:8, :], rzT[:, 4:8, 0:BL],
                                 gi0[:, s, 4:8, :])
            nc.scalar.activation(rz[:], rz[:], AF.Tanh, scale=0.5)
            nT = scr.tile([128, 8, 32], F32, tag="nT", name="nT")
            transp(nT, pgB, 4)
            gru_tail(0, rz, nT, HT[0][:])

            # ---------- GRU1 ----------
            def gru_l(l, cbprev, hbl):
                # issue k-tiles whose stationary is already available (the
                # recurrent h state) first, then the cell-dependent ones
                w = wcat[l]
                pa = psG.tile([32, 8, 4, 32], F32, tag="pg", name="pg")
                pb = psG.tile([32, 8, 4, 32], F32, tag="pg", name="pg")
                for c in (0, 1):                      # rz chunks, h k-tiles
                    for j, u in enumerate(range(4, 8)):
                        nc.tensor.matmul(
                            pa[0:BL, 4 * c:4 * (c + 1), :, :],
                            hbl[:, u - 4, :], w[:, u, 512 * c:512 * (c + 1)],
                            start=(j == 0), stop=False)
                for j, u in enumerate(range(4, 8)):   # ghn chunk (h only)
                    nc.tensor.matmul(
                        pb[0:BL, 4:8, :, :],
                        hbl[:, u - 4, :], w[:, u, 1024:1536],
                        start=(j == 0), stop=(j == 3))
                for c in (0, 1):                      # rz chunks, c k-tiles
                    for j, u in enumerate(range(0, 4)):
                        nc.tensor.matmul(
                            pa[0:BL, 4 * c:4 * (c + 1), :, :],
                            cbprev[:, u, :], w[:, u, 512 * c:512 * (c + 1)],
                            start=False, stop=(j == 3))
                for j, u in enumerate(range(0, 4)):   # gin chunk (c only)
                    nc.tensor.matmul(
                        pb[0:BL, 0:4, :, :],
                        cbprev[:, u, :], w[:, u, 1024:1536],
                        start=(j == 0), stop=(j == 3))
                return pa, pb

            def gru_math(l, pa, pb, ht_ap):
                rzTl = scr.tile([128, 8, 32], F32, tag="rzT", name="rzT")
                transp(rzTl, pa, 4, 0)
                rzl = scr.tile([128, 8, BL], F32, tag="rz", name="rz")
                nc.vector.tensor_add(rzl[:, 0:4, :], rzTl[:, 0:4, 0:BL],
                                     bcb(brz[l][:, 0:4]))
                transp(rzTl, pa, 4, 4)
                nc.vector.tensor_add(rzl[:, 4:8, :], rzTl[:, 4:8, 0:BL],
                                     bcb(brz[l][:, 4:8]))
                nc.scalar.activation(rzl[:], rzl[:], AF.Tanh, scale=0.5)
                nTl = scr.tile([128, 8, 32], F32, tag="nT", name="nT")
                transp(nTl, pb, 8)
                gru_tail(l, rzl, nTl, ht_ap)

            pa1, pb1 = gru_l(1, CB[0], HB[1])
            # MLP0 w1 matmuls can run under GRU1 math
            pm0 = psM.tile([32, 4, 4, 32], F32, tag="pm", name="pm")
            for u in range(NU):
                nc.tensor.matmul(pm0[0:BL, :, :, :], CB[0][:, u, :],
                                 mw[(1, 0)][:, u, :], start=(u == 0),
                                 stop=(u == 3))
            gru_math(1, pa1, pb1, HT[1][:])

            pa2, pb2 = gru_l(2, CB[1], hb2)

            # MLP0 w1 math + w2 matmuls under GRU2 math
            def mlp_stage(pm, wl, l):
                mt = scr.tile([128, 4, 32], F32, tag="mt", name="mt")
                transp(mt, pm, 4)
                hb_ = scr.tile([128, NU, BL], BF, tag=f"hb{wl}_{l}",
                               name=f"hb{wl}_{l}")
                nc.vector.tensor_add(mt[:, :, 0:BL], mt[:, :, 0:BL],
                                     bcb(mb[(wl, l)][:]))
                nc.gpsimd.tensor_relu(hb_[:], mt[:, :, 0:BL])
                return hb_

            h1b0 = mlp_stage(pm0, 1, 0)
            pm0b = psM.tile([32, 4, 4, 32], F32, tag="pm", name="pm")
            for u in range(NU):
                nc.tensor.matmul(pm0b[0:BL, :, :, :], h1b0[:, u, :],
                                 mw[(2, 0)][:, u, :], start=(u == 0),
                                 stop=(u == 3))

            # MLP1 w1; the whole P path is emitted before GRU2's math so
            # its transposes don't queue behind GRU2's in the DVE FIFO --
            # CT2 isn't needed until the mixing apply
            pm1 = psM.tile([32, 4, 4, 32], F32, tag="pm", name="pm")
            for u in range(NU):
                nc.tensor.matmul(pm1[0:BL, :, :, :], CB[1][:, u, :],
                                 mw[(1, 1)][:, u, :], start=(u == 0),
                                 stop=(u == 3))

            P = scr.tile([1, 2, BL], F32, tag="P", name="P")

            h2b0 = mlp_stage(pm0b, 2, 0)
            p30 = psM.tile([1, BL], F32, tag="pm", name="p30")
            for u in range(NU):
                nc.tensor.matmul(p30[:], mw[(3, 0)][:, u, :],
                                 h2b0[:, u, :], start=(u == 0), stop=(u == 3))
            nc.scalar.activation(P[:, 0, :], p30[:], AF.Tanh,
                                 bias=mb3[:, 0:1], scale=0.5)
            nc.vector.tensor_scalar(P[:, 0, :], P[:, 0, :], 0.5, 0.5,
                                    AL.mult, AL.add)
            X = scr.tile([1, 5, BL], F32, tag="X", name="X")
            nc.scalar.activation(X[:, 0, :], P[:, 0, :], AF.Identity,
                                 bias=1.0, scale=-1.0)          # 1-a (early)

            h1b1 = mlp_stage(pm1, 1, 1)
            pm1b = psM.tile([32, 4, 4, 32], F32, tag="pm", name="pm")
            for u in range(NU):
                nc.tensor.matmul(pm1b[0:BL, :, :, :], h1b1[:, u, :],
                                 mw[(2, 1)][:, u, :], start=(u == 0),
                                 stop=(u == 3))
            h2b1 = mlp_stage(pm1b, 2, 1)
            p31 = psM.tile([1, BL], F32, tag="pm", name="p31")
            for u in range(NU):
                nc.tensor.matmul(p31[:], mw[(3, 1)][:, u, :],
                                 h2b1[:, u, :], start=(u == 0), stop=(u == 3))
            nc.scalar.activation(P[:, 1, :], p31[:], AF.Tanh,
                                 bias=mb3[:, 1:2], scale=0.5)
            nc.vector.tensor_scalar(P[:, 1, :], P[:, 1, :], 0.5, 0.5,
                                    AL.mult, AL.add)

            nc.gpsimd.tensor_copy(out_acc[:, s], P[:])

            # ---------- mixing coefficients on [1, BL] tiles ----------
            a = P[:, 0, :]
            b_ = P[:, 1, :]
            nc.vector.tensor_mul(X[:, 2, :], a, b_)              # ab
            nc.vector.tensor_sub(X[:, 1, :], a, X[:, 2, :])      # a-ab
            nc.scalar.activation(X[:, 3, :], b_, AF.Identity,
                                 bias=1.0, scale=-1.0)          # 1-b
            nc.vector.tensor_copy(X[:, 4, :], b_)                # b
            E = scr.tile([1, 5, BL], F32, tag="E", name="E")
            nc.scalar.activation(E[:], X[:], AF.Exp)
            Ssum = scr.tile([1, 2, BL], F32, tag="Ssum", name="Ssum")
            nc.vector.tensor_add(Ssum[:, 0, :], E[:, 0, :], E[:, 1, :])
            nc.vector.tensor_add(Ssum[:, 0, :], Ssum[:, 0, :], E[:, 2, :])
            nc.vector.tensor_add(Ssum[:, 1, :], E[:, 3, :], E[:, 4, :])
            nc.vector.reciprocal(Ssum[:], Ssum[:])
            Mb_ = scr.tile([1, 5, BL], BF, tag="Mb", name="Mb")
            nc.vector.tensor_mul(
                Mb_[:, 0:3, :], E[:, 0:3, :],
                Ssum[:, 0, :].unsqueeze(1).broadcast_to((1, 3, BL)))
            nc.vector.tensor_mul(
                Mb_[:, 3:5, :], E[:, 3:5, :],
                Ssum[:, 1, :].unsqueeze(1).broadcast_to((1, 2, BL)))
            pbx = psM.tile([128, 5, BL], F32, tag="pm", name="pbx")
            nc.tensor.matmul(pbx[:], onesb[:], Mb_[:],
                             start=True, stop=True)
            # ---------- mixing apply, CT0/CT1 terms ----------
            # hoisted before GRU2's math; all muls on DVE reading the
            # broadcast PSUM directly (no SBUF coefficient copy), so the
            # post-CT2 chain is same-engine with no cross-engine sems
            ta = scr.tile([128, NU, BL], F32, tag="ta", name="ta")
            tb = scr.tile([128, NU, BL], F32, tag="tb", name="tb")
            tc_ = scr.tile([128, NU, BL], F32, tag="tc", name="tc")
            td_ = scr.tile([128, NU, BL], F32, tag="td", name="td")
            nc.vector.tensor_mul(ta[:], CT[0], bcc(pbx[:, 0, :]))
            nc.vector.tensor_mul(tb[:], CT[1], bcc(pbx[:, 1, :]))
            nc.vector.tensor_add(ta[:], ta[:], tb[:])
            nc.vector.tensor_mul(tc_[:], CT[1], bcc(pbx[:, 3, :]))

            gru_math(2, pa2, pb2, ht2)

            # ---------- mixing apply, CT2 terms ----------
            nc.vector.tensor_mul(tb[:], CT[2], bcc(pbx[:, 2, :]))
            nc.vector.tensor_add(HB[0][:], ta[:], tb[:])
            nc.gpsimd.tensor_add(HT[0][:], ta[:], tb[:])
            nc.vector.tensor_mul(td_[:], CT[2], bcc(pbx[:, 4, :]))
            nc.vector.tensor_add(HB[1][:], tc_[:], td_[:])
            nc.gpsimd.tensor_add(HT[1][:], tc_[:], td_[:])
            ct2_prev, cb2_prev = CT[2], CB[2]

        nc.gpsimd.dma_start(pout[:], out_acc[:])

    nc.finalize()
    return nc


def _kernel_numpy(np_inputs, t_steps):
    W = prep_host(np_inputs)
    outs = [emulate_core(W, np_inputs["x"][BL * c:BL * (c + 1)].astype(
        np.float32), t_steps=t_steps) for c in range(NCORES)]
    return np.concatenate(outs, axis=0).astype(np.float32)


def kernel(_t_steps=T, **inputs):
    np_inputs = {k: np.asarray(v) for k, v in inputs.items()}
    if os.environ.get("HRNN_NUMPY"):
        return _kernel_numpy(np_inputs, _t_steps)
    return _kernel_bass(np_inputs, _t_steps)


def _kernel_bass(np_inputs, _t_steps):
    W = prep_host(np_inputs)
    t_steps = _t_steps
    key = t_steps
    if key not in _CACHE:
        _CACHE[key] = build_program(t_steps)
    nc = _CACHE[key]
    shared = {k: v for k, v in W.items()
              if k not in ("mb3_0", "mb3_1")}
    # P path computes sigmoid as 0.5*(1+tanh((x+b)/2)) -> bias input is b/2
    mb3 = np.array([[0.5 * W["mb3_0"], 0.5 * W["mb3_1"]]], np.float32)
    x = np_inputs["x"].astype(np.float32)
    in_maps = []
    for c in range(NCORES):
        m = dict(shared)
        m["mb3"] = mb3
        m["xT"] = prep_x(x[BL * c:BL * (c + 1)])
        in_maps.append(m)
    from concourse.bass_utils import run_bass_kernel_spmd
    res = run_bass_kernel_spmd(nc, in_maps, core_ids=list(range(NCORES)))
    outs = []
    for c in range(NCORES):
        p = res.results[c]["pout"].reshape(t_steps, 2, BL)
        outs.append(p.transpose(2, 0, 1))
    return np.concatenate(outs, axis=0).astype(np.float32)


if __name__ == "__main__":
    pass
